# revision 3
# baseline (speedup 1.0000x reference)
"""HaarWavelet2D (level=2) Trainium2 kernel.

Contract: kernel(x, level) with x [8, 64, 256, 256] fp32, level=2.
Returns (low_freq, high_freq), each [8, 64, 256, 256] fp32 — matching the
jax reference (2-level Haar decomposition with bilinear resizes).

Sharding: data-parallel over the batch dim — core b processes x[b] (64
channels). Inside each core, channels are processed in groups of G=2 with
rows in partitions (even/odd row-parity tiles), columns*channels in the
free dimension.

Math (validated vs reference in model.py):
  s = x[:,j]+x[:,j+1]; d = x[:,j]-x[:,j+1]
  t1 = s[r]-s[r+1];   m = max(|d[r]|,|d[r+1]|)       (abs_max ALU op)
  ch0 = 0.5|t1| + m   (== 2*(|lh|+|hl|+|hh|) of level 0)
  Y_L = Va @ s        (Va = 0.25*V255@Sv1 — vertical resize+pair-sum fold)
  Y_h = (0.25*V255) @ ch0
  L0 = Rh255(Y_L); h0 = Rh255(Y_h)                   (horizontal resize)
  level 1 on L0 via stride-2 column pairs + row-parity tiles, V128 resize
  high = h0 + h1; low = Rh128(V128q @ lsum1)
All vertical linear ops run on the tensor engine as banded-matrix matmuls
(bf16 weights, fp32 PSUM); horizontal resizes use the pad+diff trick on
the vector engine; scalar/gpsimd engines do casts and shifted copies.
"""

import sys

if "/opt/trn_rl_repo" not in sys.path:
    sys.path.insert(0, "/opt/trn_rl_repo")

import numpy as np
import ml_dtypes

BF = ml_dtypes.bfloat16

B_, C_, H_, W_ = 8, 64, 256, 256
NCORES = 8
G = 2  # channels per inner iteration


# ----------------------------------------------------------------------------
# host-side weight construction
# ----------------------------------------------------------------------------

def _resize_matrix(n, N):
    M = np.zeros((N, n), dtype=np.float64)
    for i in range(N):
        c = (i + 0.5) * n / N - 0.5
        j0 = int(np.floor(c))
        f = c - j0
        M[i, min(max(j0, 0), n - 1)] += 1.0 - f
        M[i, min(max(j0 + 1, 0), n - 1)] += f
    return M


def _build_weights():
    V255 = _resize_matrix(255, 256)
    V128 = _resize_matrix(128, 256)
    Sv1 = np.zeros((255, 256))
    for r in range(255):
        Sv1[r, r] = 1.0
        Sv1[r, r + 1] = 1.0

    Va = 0.25 * (V255 @ Sv1)      # [256, 256]
    V255s = 0.25 * V255           # [256, 255]
    V128q = 0.25 * V128           # [256, 128]
    W0 = np.array([V255[i, i - 1] if i >= 1 else 0.0 for i in range(256)])

    w = {
        # L0 vertical: Y_L(parity p rows) = Va[p::2, 0::2] @ sE + Va[p::2, 1::2] @ sO
        "w_va_ee": Va[0::2, 0::2].T,   # [128,128]
        "w_va_eo": Va[0::2, 1::2].T,
        "w_va_oe": Va[1::2, 0::2].T,
        "w_va_oo": Va[1::2, 1::2].T,
        # h0 vertical: rows 0:128 (A) / 128:256 (B); ch0 rows even(128)/odd(127)
        "w_vh_ae": V255s[0:128, 0::2].T,   # [128,128]
        "w_vh_ao": V255s[0:128, 1::2].T,   # [127,128]
        "w_vh_be": V255s[128:256, 0::2].T,
        "w_vh_bo": V255s[128:256, 1::2].T,
        # level-1 vertical
        "w_vq_a": V128q[0:128, :].T,   # [128,128]
        "w_vq_b": V128q[128:256, :].T,
        # horizontal 255->256 weights, replicated over partitions
        "w0t": np.tile(W0[None, :], (128, 1)),   # [128,256]
    }
    return {k: v.astype(BF) for k, v in w.items()}


_WEIGHTS = None


def _weights():
    global _WEIGHTS
    if _WEIGHTS is None:
        _WEIGHTS = _build_weights()
    return _WEIGHTS


# ----------------------------------------------------------------------------
# bass program
# ----------------------------------------------------------------------------

_NC_CACHE = {}


def build_nc(C=C_):
    key = C
    if key in _NC_CACHE:
        return _NC_CACHE[key]

    import concourse.bass as bass
    import concourse.bacc as bacc
    import concourse.tile as tile
    import concourse.mybir as mybir

    F32 = mybir.dt.float32
    BF16 = mybir.dt.bfloat16
    Alu = mybir.AluOpType
    Act = mybir.ActivationFunctionType
    P = 128

    nc = bacc.Bacc("TRN2", target_bir_lowering=False)
    x_d = nc.dram_tensor("x", [C, H_, W_], F32, kind="ExternalInput")
    wt = _weights()
    w_d = {
        name: nc.dram_tensor(name, list(arr.shape), BF16, kind="ExternalInput")
        for name, arr in wt.items()
    }
    low_d = nc.dram_tensor("low", [C, H_, W_], F32, kind="ExternalOutput")
    high_d = nc.dram_tensor("high", [C, H_, W_], F32, kind="ExternalOutput")

    def bcast_cols(ap, g):
        # weight AP [128, N] -> [128, (0,g), N]: repeat per channel group
        return bass.AP(tensor=ap.tensor, offset=ap.offset,
                       ap=[ap.ap[0], [0, g], ap.ap[1]])

    with tile.TileContext(nc) as tc:
        with (
            tc.tile_pool(name="consts", bufs=1) as consts,
            tc.tile_pool(name="xin", bufs=2) as xin,
            tc.tile_pool(name="sd", bufs=2) as sd,
            tc.tile_pool(name="mid", bufs=2) as mid,
            tc.tile_pool(name="hor", bufs=2) as hor,
            tc.tile_pool(name="lv1", bufs=2) as lv1,
            tc.tile_pool(name="outp", bufs=2) as outp,
            tc.tile_pool(name="ps0", bufs=1, space="PSUM") as ps0,
            tc.tile_pool(name="ps1", bufs=1, space="PSUM") as ps1,
        ):
            wtile = {}
            for name, arr in wt.items():
                t = consts.tile(list(arr.shape), BF16, tag=name)
                nc.sync.dma_start(out=t, in_=w_d[name][:, :])
                wtile[name] = t

            def rh255(Y, out_name):
                """Horizontal 255->256 resize of PSUM tile Y [128,G,255].
                Returns bf16 SBUF tile [128, G, 256]."""
                q = hor.tile([P, G, 256], BF16, tag=f"q_{out_name}")
                nc.scalar.copy(out=q[:, :, 0:255], in_=Y)
                nc.scalar.copy(out=q[:, :, 255:256], in_=Y[:, :, 254:255])
                q1 = hor.tile([P, G, 256], BF16, tag=f"q1_{out_name}")
                nc.gpsimd.tensor_copy(out=q1[:, :, 1:256], in_=q[:, :, 0:255])
                nc.gpsimd.tensor_copy(out=q1[:, :, 0:1], in_=q[:, :, 0:1])
                diff = hor.tile([P, G, 256], BF16, tag=f"df_{out_name}")
                nc.vector.tensor_tensor(out=diff, in0=q1, in1=q, op=Alu.subtract)
                mult = hor.tile([P, G, 256], BF16, tag=f"mu_{out_name}")
                nc.vector.tensor_tensor(out=mult, in0=diff,
                                        in1=bcast_cols(wtile["w0t"][:, :], G),
                                        op=Alu.mult)
                out = hor.tile([P, G, 256], BF16, tag=out_name)
                nc.vector.tensor_tensor(out=out, in0=q, in1=mult, op=Alu.add)
                return out

            n_iter = C // G
            for it in range(n_iter):
                c0 = it * G

                # ---- load x row-parity tiles -------------------------------
                xE = xin.tile([P, G, W_], F32, tag="xE")
                xO = xin.tile([P, G, W_], F32, tag="xO")
                nc.sync.dma_start(
                    out=xE, in_=x_d[c0:c0 + G, 0:H_:2, :].rearrange("c r w -> r c w"))
                nc.sync.dma_start(
                    out=xO, in_=x_d[c0:c0 + G, 1:H_:2, :].rearrange("c r w -> r c w"))

                # ---- level-0 horizontal pair sum/diff ----------------------
                sE = sd.tile([P, G, 255], BF16, tag="sE")
                sO = sd.tile([P, G, 255], BF16, tag="sO")
                dE = sd.tile([P, G, 255], BF16, tag="dE")
                dO = sd.tile([P, G, 255], BF16, tag="dO")
                nc.vector.tensor_tensor(out=sE, in0=xE[:, :, 0:255], in1=xE[:, :, 1:256], op=Alu.add)
                nc.vector.tensor_tensor(out=sO, in0=xO[:, :, 0:255], in1=xO[:, :, 1:256], op=Alu.add)
                nc.vector.tensor_tensor(out=dE, in0=xE[:, :, 0:255], in1=xE[:, :, 1:256], op=Alu.subtract)
                nc.vector.tensor_tensor(out=dO, in0=xO[:, :, 0:255], in1=xO[:, :, 1:256], op=Alu.subtract)
                # |d| on the scalar engine (abs_max is not supported by codegen)
                adE = sd.tile([P, G, 255], BF16, tag="adE")
                adO = sd.tile([P, G, 255], BF16, tag="adO")
                nc.scalar.activation(out=adE, in_=dE, func=Act.Abs)
                nc.scalar.activation(out=adO, in_=dO, func=Act.Abs)
                # shifted copies (rows 2,4..254) via SBUF->SBUF DMA
                sE2 = sd.tile([127, G, 255], BF16, tag="sE2")
                adE2 = sd.tile([127, G, 255], BF16, tag="adE2")
                nc.sync.dma_start(out=sE2, in_=sE[1:128, :, :])
                nc.sync.dma_start(out=adE2, in_=adE[1:128, :, :])

                # ---- level-0 vertical pair ops -----------------------------
                t1E = mid.tile([P, G, 255], BF16, tag="t1E")
                t1O = mid.tile([127, G, 255], BF16, tag="t1O")
                mE = mid.tile([P, G, 255], BF16, tag="mE")
                mO = mid.tile([127, G, 255], BF16, tag="mO")
                nc.vector.tensor_tensor(out=t1E, in0=sE, in1=sO, op=Alu.subtract)
                nc.vector.tensor_tensor(out=t1O, in0=sO[0:127, :, :], in1=sE2, op=Alu.subtract)
                nc.vector.tensor_tensor(out=mE, in0=adE, in1=adO, op=Alu.max)
                nc.vector.tensor_tensor(out=mO, in0=adO[0:127, :, :], in1=adE2, op=Alu.max)

                a1E = mid.tile([P, G, 255], BF16, tag="a1E")
                a1O = mid.tile([127, G, 255], BF16, tag="a1O")
                nc.scalar.activation(out=a1E, in_=t1E, func=Act.Abs, scale=0.5)
                nc.scalar.activation(out=a1O, in_=t1O, func=Act.Abs, scale=0.5)
                ch0E = mid.tile([P, G, 255], BF16, tag="ch0E")
                ch0O = mid.tile([127, G, 255], BF16, tag="ch0O")
                nc.vector.tensor_tensor(out=ch0E, in0=a1E, in1=mE, op=Alu.add)
                nc.vector.tensor_tensor(out=ch0O, in0=a1O, in1=mO, op=Alu.add)

                # ---- level-0 vertical matmuls ------------------------------
                Y_Le = ps0.tile([P, G, 255], F32, tag="Y_Le")
                Y_Lo = ps0.tile([P, G, 255], F32, tag="Y_Lo")
                nc.tensor.matmul(out=Y_Le, lhsT=wtile["w_va_ee"][:, :], rhs=sE, start=True, stop=False)
                nc.tensor.matmul(out=Y_Le, lhsT=wtile["w_va_eo"][:, :], rhs=sO, start=False, stop=True)
                nc.tensor.matmul(out=Y_Lo, lhsT=wtile["w_va_oe"][:, :], rhs=sE, start=True, stop=False)
                nc.tensor.matmul(out=Y_Lo, lhsT=wtile["w_va_oo"][:, :], rhs=sO, start=False, stop=True)
                Y_hA = ps0.tile([P, G, 255], F32, tag="Y_hA")
                Y_hB = ps0.tile([P, G, 255], F32, tag="Y_hB")
                nc.tensor.matmul(out=Y_hA, lhsT=wtile["w_vh_ae"][:, :], rhs=ch0E, start=True, stop=False)
                nc.tensor.matmul(out=Y_hA, lhsT=wtile["w_vh_ao"][:, :], rhs=ch0O, start=False, stop=True)
                nc.tensor.matmul(out=Y_hB, lhsT=wtile["w_vh_be"][:, :], rhs=ch0E, start=True, stop=False)
                nc.tensor.matmul(out=Y_hB, lhsT=wtile["w_vh_bo"][:, :], rhs=ch0O, start=False, stop=True)

                # ---- level-0 horizontal resizes ----------------------------
                L0e = rh255(Y_Le, "L0e")
                L0o = rh255(Y_Lo, "L0o")
                h0A = rh255(Y_hA, "h0A")
                h0B = rh255(Y_hB, "h0B")

                # ---- level-1 elementwise -----------------------------------
                s2e = lv1.tile([P, G, 128], BF16, tag="s2e")
                s2o = lv1.tile([P, G, 128], BF16, tag="s2o")
                d2e = lv1.tile([P, G, 128], BF16, tag="d2e")
                d2o = lv1.tile([P, G, 128], BF16, tag="d2o")
                nc.vector.tensor_tensor(out=s2e, in0=L0e[:, :, 0:256:2], in1=L0e[:, :, 1:256:2], op=Alu.add)
                nc.vector.tensor_tensor(out=s2o, in0=L0o[:, :, 0:256:2], in1=L0o[:, :, 1:256:2], op=Alu.add)
                nc.vector.tensor_tensor(out=d2e, in0=L0e[:, :, 0:256:2], in1=L0e[:, :, 1:256:2], op=Alu.subtract)
                nc.vector.tensor_tensor(out=d2o, in0=L0o[:, :, 0:256:2], in1=L0o[:, :, 1:256:2], op=Alu.subtract)
                lsum1 = lv1.tile([P, G, 128], BF16, tag="lsum1")
                t1b = lv1.tile([P, G, 128], BF16, tag="t1b")
                ad2e = lv1.tile([P, G, 128], BF16, tag="ad2e")
                ad2o = lv1.tile([P, G, 128], BF16, tag="ad2o")
                m1 = lv1.tile([P, G, 128], BF16, tag="m1")
                nc.vector.tensor_tensor(out=lsum1, in0=s2e, in1=s2o, op=Alu.add)
                nc.vector.tensor_tensor(out=t1b, in0=s2e, in1=s2o, op=Alu.subtract)
                nc.scalar.activation(out=ad2e, in_=d2e, func=Act.Abs)
                nc.scalar.activation(out=ad2o, in_=d2o, func=Act.Abs)
                nc.vector.tensor_tensor(out=m1, in0=ad2e, in1=ad2o, op=Alu.max)
                a1b = lv1.tile([P, G, 128], BF16, tag="a1b")
                nc.scalar.activation(out=a1b, in_=t1b, func=Act.Abs, scale=0.5)
                ch1 = lv1.tile([P, G, 128], BF16, tag="ch1")
                nc.vector.tensor_tensor(out=ch1, in0=a1b, in1=m1, op=Alu.add)

                # ---- level-1 vertical matmuls ------------------------------
                Y_lo = ps1.tile([P, 2, G, 128], F32, tag="Y_lo")
                Y_h1 = ps1.tile([P, 2, G, 128], F32, tag="Y_h1")
                nc.tensor.matmul(out=Y_lo[:, 0], lhsT=wtile["w_vq_a"][:, :], rhs=lsum1, start=True, stop=True)
                nc.tensor.matmul(out=Y_lo[:, 1], lhsT=wtile["w_vq_b"][:, :], rhs=lsum1, start=True, stop=True)
                nc.tensor.matmul(out=Y_h1[:, 0], lhsT=wtile["w_vq_a"][:, :], rhs=ch1, start=True, stop=True)
                nc.tensor.matmul(out=Y_h1[:, 1], lhsT=wtile["w_vq_b"][:, :], rhs=ch1, start=True, stop=True)

                # ---- level-1 horizontal (128->256) + finalization ----------
                def rh128(Y, name):
                    """Y: PSUM [128, 2, G, 128] -> (ev, od) bf16 [128,2,G,128]."""
                    q = lv1.tile([P, 2, G, 129], BF16, tag=f"q_{name}")
                    nc.scalar.copy(out=q[:, :, :, 0:128], in_=Y)
                    nc.scalar.copy(out=q[:, :, :, 128:129], in_=Y[:, :, :, 127:128])
                    q1 = lv1.tile([P, 2, G, 129], BF16, tag=f"q1_{name}")
                    nc.gpsimd.tensor_copy(out=q1[:, :, :, 1:129], in_=q[:, :, :, 0:128])
                    nc.gpsimd.tensor_copy(out=q1[:, :, :, 0:1], in_=q[:, :, :, 0:1])
                    diff = lv1.tile([P, 2, G, 129], BF16, tag=f"df_{name}")
                    nc.vector.tensor_tensor(out=diff, in0=q1, in1=q, op=Alu.subtract)
                    ev = lv1.tile([P, 2, G, 128], BF16, tag=f"ev_{name}")
                    od = lv1.tile([P, 2, G, 128], BF16, tag=f"od_{name}")
                    nc.vector.scalar_tensor_tensor(
                        out=ev, in0=diff[:, :, :, 0:128], scalar=0.25,
                        in1=q[:, :, :, 0:128], op0=Alu.mult, op1=Alu.add)
                    nc.vector.scalar_tensor_tensor(
                        out=od, in0=diff[:, :, :, 1:129], scalar=-0.25,
                        in1=q[:, :, :, 0:128], op0=Alu.mult, op1=Alu.add)
                    return ev, od

                lo_ev, lo_od = rh128(Y_lo, "lo")
                h1_ev, h1_od = rh128(Y_h1, "h1")

                lowA = outp.tile([P, G, W_], F32, tag="lowA")
                lowB = outp.tile([P, G, W_], F32, tag="lowB")
                nc.gpsimd.tensor_copy(out=lowA[:, :, 0:256:2], in_=lo_ev[:, 0])
                nc.gpsimd.tensor_copy(out=lowA[:, :, 1:256:2], in_=lo_od[:, 0])
                nc.gpsimd.tensor_copy(out=lowB[:, :, 0:256:2], in_=lo_ev[:, 1])
                nc.gpsimd.tensor_copy(out=lowB[:, :, 1:256:2], in_=lo_od[:, 1])

                highA = outp.tile([P, G, W_], F32, tag="highA")
                highB = outp.tile([P, G, W_], F32, tag="highB")
                nc.vector.tensor_tensor(out=highA[:, :, 0:256:2], in0=h1_ev[:, 0],
                                        in1=h0A[:, :, 0:256:2], op=Alu.add)
                nc.vector.tensor_tensor(out=highA[:, :, 1:256:2], in0=h1_od[:, 0],
                                        in1=h0A[:, :, 1:256:2], op=Alu.add)
                nc.vector.tensor_tensor(out=highB[:, :, 0:256:2], in0=h1_ev[:, 1],
                                        in1=h0B[:, :, 0:256:2], op=Alu.add)
                nc.vector.tensor_tensor(out=highB[:, :, 1:256:2], in0=h1_od[:, 1],
                                        in1=h0B[:, :, 1:256:2], op=Alu.add)

                # ---- store --------------------------------------------------
                nc.sync.dma_start(
                    out=low_d[c0:c0 + G, 0:128, :].rearrange("c r w -> r c w"), in_=lowA)
                nc.sync.dma_start(
                    out=low_d[c0:c0 + G, 128:256, :].rearrange("c r w -> r c w"), in_=lowB)
                nc.sync.dma_start(
                    out=high_d[c0:c0 + G, 0:128, :].rearrange("c r w -> r c w"), in_=highA)
                nc.sync.dma_start(
                    out=high_d[c0:c0 + G, 128:256, :].rearrange("c r w -> r c w"), in_=highB)

    nc.compile()
    _NC_CACHE[key] = nc
    return nc


# ----------------------------------------------------------------------------
# host entry points
# ----------------------------------------------------------------------------

def _run_device(x, trace=False):
    """x: [8, 64, 256, 256] fp32. Returns (low, high, results_obj)."""
    from concourse import bass_utils

    nc = build_nc(C_)
    wt = _weights()
    in_maps = [dict(wt, x=np.ascontiguousarray(x[b])) for b in range(NCORES)]
    res = bass_utils.run_bass_kernel_spmd(
        nc, in_maps, core_ids=list(range(NCORES)), trace=trace)
    low = np.stack([res.results[b]["low"] for b in range(NCORES)])
    high = np.stack([res.results[b]["high"] for b in range(NCORES)])
    return low, high, res


def _fallback(x, level):
    """Numpy port of the reference for unexpected shapes/levels."""
    xl = x.astype(np.float64)
    Bb, Cc, H, W = xl.shape
    low = xl
    high = np.zeros_like(xl)

    def up(a, n_r, n_c):
        Mr = _resize_matrix(a.shape[-2], n_r)
        Mc = _resize_matrix(a.shape[-1], n_c)
        return np.einsum("ij,...jk,lk->...il", Mr, a, Mc)

    for lv in range(level):
        stride = 2 ** lv
        if H // stride < 2 or W // stride < 2:
            break
        x00 = low[..., 0:H - 1:stride, 0:W - 1:stride]
        x01 = low[..., 0:H - 1:stride, 1:W:stride]
        x10 = low[..., 1:H:stride, 0:W - 1:stride]
        x11 = low[..., 1:H:stride, 1:W:stride]
        ll = (x00 + x01 + x10 + x11) * 0.25
        lh = (x00 + x01 - x10 - x11) * 0.25
        hl = (x00 - x01 + x10 - x11) * 0.25
        hh = (x00 - x01 - x10 + x11) * 0.25
        ch = np.abs(lh) + np.abs(hl) + np.abs(hh)
        high = high + up(ch, H, W)
        low = up(ll, H, W)
    if level > 0:
        high = high / level
    return low.astype(np.float32), high.astype(np.float32)


def kernel(x, level):
    x = np.asarray(x, dtype=np.float32)
    level = int(level)
    if level != 2 or x.shape != (B_, C_, H_, W_):
        return _fallback(x, level)
    low, high, _ = _run_device(x)
    return low, high


# revision 4
# speedup vs baseline: 1218.9617x; 1218.9617x over previous
"""HaarWavelet2D (level=2) Trainium2 kernel.

Contract: kernel(x, level) with x [8, 64, 256, 256] fp32, level=2.
Returns (low_freq, high_freq), each [8, 64, 256, 256] fp32 — matching the
jax reference (2-level Haar decomposition with bilinear resizes).

Sharding: data-parallel over the batch dim — core b processes x[b] (64
channels). Inside each core, channels are processed in groups of G=2 with
rows in partitions (even/odd row-parity tiles), columns*channels in the
free dimension.

Math (validated vs reference in model.py):
  s = x[:,j]+x[:,j+1]; d = x[:,j]-x[:,j+1]
  t1 = s[r]-s[r+1];   m = max(|d[r]|,|d[r+1]|)       (abs_max ALU op)
  ch0 = 0.5|t1| + m   (== 2*(|lh|+|hl|+|hh|) of level 0)
  Y_L = Va @ s        (Va = 0.25*V255@Sv1 — vertical resize+pair-sum fold)
  Y_h = (0.25*V255) @ ch0
  L0 = Rh255(Y_L); h0 = Rh255(Y_h)                   (horizontal resize)
  level 1 on L0 via stride-2 column pairs + row-parity tiles, V128 resize
  high = h0 + h1; low = Rh128(V128q @ lsum1)
All vertical linear ops run on the tensor engine as banded-matrix matmuls
(bf16 weights, fp32 PSUM); horizontal resizes use the pad+diff trick on
the vector engine; scalar/gpsimd engines do casts and shifted copies.
"""

import sys

if "/opt/trn_rl_repo" not in sys.path:
    sys.path.insert(0, "/opt/trn_rl_repo")

import numpy as np
import ml_dtypes

BF = ml_dtypes.bfloat16

B_, C_, H_, W_ = 8, 64, 256, 256
NCORES = 8
G = 2  # channels per inner iteration


# ----------------------------------------------------------------------------
# host-side weight construction
# ----------------------------------------------------------------------------

def _resize_matrix(n, N):
    M = np.zeros((N, n), dtype=np.float64)
    for i in range(N):
        c = (i + 0.5) * n / N - 0.5
        j0 = int(np.floor(c))
        f = c - j0
        M[i, min(max(j0, 0), n - 1)] += 1.0 - f
        M[i, min(max(j0 + 1, 0), n - 1)] += f
    return M


def _build_weights():
    V255 = _resize_matrix(255, 256)
    V128 = _resize_matrix(128, 256)
    Sv1 = np.zeros((255, 256))
    for r in range(255):
        Sv1[r, r] = 1.0
        Sv1[r, r + 1] = 1.0

    Va = 0.25 * (V255 @ Sv1)      # [256, 256]
    V255s = 0.25 * V255           # [256, 255]
    V128q = 0.25 * V128           # [256, 128]
    W0 = np.array([V255[i, i - 1] if i >= 1 else 0.0 for i in range(256)])

    w = {
        # L0 vertical: Y_L(parity p rows) = Va[p::2, 0::2] @ sE + Va[p::2, 1::2] @ sO
        "w_va_ee": Va[0::2, 0::2].T,   # [128,128]
        "w_va_eo": Va[0::2, 1::2].T,
        "w_va_oe": Va[1::2, 0::2].T,
        "w_va_oo": Va[1::2, 1::2].T,
        # h0 vertical: rows 0:128 (A) / 128:256 (B); ch0 rows even(128)/odd(127)
        "w_vh_ae": V255s[0:128, 0::2].T,   # [128,128]
        "w_vh_ao": V255s[0:128, 1::2].T,   # [127,128]
        "w_vh_be": V255s[128:256, 0::2].T,
        "w_vh_bo": V255s[128:256, 1::2].T,
        # level-1 vertical
        "w_vq_a": V128q[0:128, :].T,   # [128,128]
        "w_vq_b": V128q[128:256, :].T,
        # horizontal 255->256 weights, replicated over partitions
        "w0t": np.tile(W0[None, :], (128, 1)),   # [128,256]
    }
    return {k: v.astype(BF) for k, v in w.items()}


_WEIGHTS = None


def _weights():
    global _WEIGHTS
    if _WEIGHTS is None:
        _WEIGHTS = _build_weights()
    return _WEIGHTS


# ----------------------------------------------------------------------------
# bass program
# ----------------------------------------------------------------------------

_NC_CACHE = {}


def build_nc(C=C_):
    key = C
    if key in _NC_CACHE:
        return _NC_CACHE[key]

    import concourse.bass as bass
    import concourse.bacc as bacc
    import concourse.tile as tile
    import concourse.mybir as mybir

    F32 = mybir.dt.float32
    BF16 = mybir.dt.bfloat16
    Alu = mybir.AluOpType
    Act = mybir.ActivationFunctionType
    P = 128

    nc = bacc.Bacc("TRN2", target_bir_lowering=False)
    x_d = nc.dram_tensor("x", [C, H_, W_], F32, kind="ExternalInput")
    wt = _weights()
    w_d = {
        name: nc.dram_tensor(name, list(arr.shape), BF16, kind="ExternalInput")
        for name, arr in wt.items()
    }
    low_d = nc.dram_tensor("low", [C, H_, W_], F32, kind="ExternalOutput")
    high_d = nc.dram_tensor("high", [C, H_, W_], F32, kind="ExternalOutput")

    def bcast_cols(ap, g):
        # weight AP [128, N] -> [128, (0,g), N]: repeat per channel group
        return bass.AP(tensor=ap.tensor, offset=ap.offset,
                       ap=[ap.ap[0], [0, g], ap.ap[1]])

    with tile.TileContext(nc) as tc:
        with (
            tc.tile_pool(name="consts", bufs=1) as consts,
            tc.tile_pool(name="xin", bufs=2) as xin,
            tc.tile_pool(name="sd", bufs=2) as sd,
            tc.tile_pool(name="mid", bufs=2) as mid,
            tc.tile_pool(name="hor", bufs=2) as hor,
            tc.tile_pool(name="lv1", bufs=2) as lv1,
            tc.tile_pool(name="outp", bufs=2) as outp,
            tc.tile_pool(name="ps0", bufs=1, space="PSUM") as ps0,
            tc.tile_pool(name="ps1", bufs=1, space="PSUM") as ps1,
        ):
            wtile = {}
            for name, arr in wt.items():
                t = consts.tile(list(arr.shape), BF16, tag=name)
                nc.sync.dma_start(out=t, in_=w_d[name][:, :])
                wtile[name] = t

            def rh255(Y, out_name):
                """Horizontal 255->256 resize of PSUM tile Y [128,G,255].
                Returns bf16 SBUF tile [128, G, 256]."""
                q = hor.tile([P, G, 256], BF16, tag=f"q_{out_name}")
                nc.scalar.copy(out=q[:, :, 0:255], in_=Y)
                nc.scalar.copy(out=q[:, :, 255:256], in_=Y[:, :, 254:255])
                q1 = hor.tile([P, G, 256], BF16, tag=f"q1_{out_name}")
                nc.gpsimd.tensor_copy(out=q1[:, :, 1:256], in_=q[:, :, 0:255])
                nc.gpsimd.tensor_copy(out=q1[:, :, 0:1], in_=q[:, :, 0:1])
                diff = hor.tile([P, G, 256], BF16, tag=f"df_{out_name}")
                nc.vector.tensor_tensor(out=diff, in0=q1, in1=q, op=Alu.subtract)
                mult = hor.tile([P, G, 256], BF16, tag=f"mu_{out_name}")
                nc.vector.tensor_tensor(out=mult, in0=diff,
                                        in1=bcast_cols(wtile["w0t"][:, :], G),
                                        op=Alu.mult)
                out = hor.tile([P, G, 256], BF16, tag=out_name)
                nc.vector.tensor_tensor(out=out, in0=q, in1=mult, op=Alu.add)
                return out

            n_iter = C // G
            for it in range(n_iter):
                c0 = it * G

                # ---- load x row-parity tiles -------------------------------
                xE = xin.tile([P, G, W_], F32, tag="xE")
                xO = xin.tile([P, G, W_], F32, tag="xO")
                nc.sync.dma_start(
                    out=xE, in_=x_d[c0:c0 + G, 0:H_:2, :].rearrange("c r w -> r c w"))
                nc.sync.dma_start(
                    out=xO, in_=x_d[c0:c0 + G, 1:H_:2, :].rearrange("c r w -> r c w"))

                # ---- level-0 horizontal pair sum/diff ----------------------
                sE = sd.tile([P, G, 255], BF16, tag="sE")
                sO = sd.tile([P, G, 255], BF16, tag="sO")
                dE = sd.tile([P, G, 255], BF16, tag="dE")
                dO = sd.tile([P, G, 255], BF16, tag="dO")
                nc.vector.tensor_tensor(out=sE, in0=xE[:, :, 0:255], in1=xE[:, :, 1:256], op=Alu.add)
                nc.vector.tensor_tensor(out=sO, in0=xO[:, :, 0:255], in1=xO[:, :, 1:256], op=Alu.add)
                nc.vector.tensor_tensor(out=dE, in0=xE[:, :, 0:255], in1=xE[:, :, 1:256], op=Alu.subtract)
                nc.vector.tensor_tensor(out=dO, in0=xO[:, :, 0:255], in1=xO[:, :, 1:256], op=Alu.subtract)
                # |d| on the scalar engine (abs_max is not supported by codegen)
                adE = sd.tile([P, G, 255], BF16, tag="adE")
                adO = sd.tile([P, G, 255], BF16, tag="adO")
                nc.scalar.activation(out=adE, in_=dE, func=Act.Abs)
                nc.scalar.activation(out=adO, in_=dO, func=Act.Abs)
                # shifted copies (rows 2,4..254) via SBUF->SBUF DMA
                sE2 = sd.tile([127, G, 255], BF16, tag="sE2")
                adE2 = sd.tile([127, G, 255], BF16, tag="adE2")
                nc.sync.dma_start(out=sE2, in_=sE[1:128, :, :])
                nc.sync.dma_start(out=adE2, in_=adE[1:128, :, :])

                # ---- level-0 vertical pair ops -----------------------------
                t1E = mid.tile([P, G, 255], BF16, tag="t1E")
                t1O = mid.tile([127, G, 255], BF16, tag="t1O")
                mE = mid.tile([P, G, 255], BF16, tag="mE")
                mO = mid.tile([127, G, 255], BF16, tag="mO")
                nc.vector.tensor_tensor(out=t1E, in0=sE, in1=sO, op=Alu.subtract)
                nc.vector.tensor_tensor(out=t1O, in0=sO[0:127, :, :], in1=sE2, op=Alu.subtract)
                nc.vector.tensor_tensor(out=mE, in0=adE, in1=adO, op=Alu.max)
                nc.vector.tensor_tensor(out=mO, in0=adO[0:127, :, :], in1=adE2, op=Alu.max)

                a1E = mid.tile([P, G, 255], BF16, tag="a1E")
                a1O = mid.tile([127, G, 255], BF16, tag="a1O")
                nc.scalar.activation(out=a1E, in_=t1E, func=Act.Abs, scale=0.5)
                nc.scalar.activation(out=a1O, in_=t1O, func=Act.Abs, scale=0.5)
                ch0E = mid.tile([P, G, 255], BF16, tag="ch0E")
                ch0O = mid.tile([127, G, 255], BF16, tag="ch0O")
                nc.vector.tensor_tensor(out=ch0E, in0=a1E, in1=mE, op=Alu.add)
                nc.vector.tensor_tensor(out=ch0O, in0=a1O, in1=mO, op=Alu.add)

                # ---- level-0 vertical matmuls ------------------------------
                Y_Le = ps0.tile([P, G, 255], F32, tag="Y_Le")
                Y_Lo = ps0.tile([P, G, 255], F32, tag="Y_Lo")
                nc.tensor.matmul(out=Y_Le, lhsT=wtile["w_va_ee"][:, :], rhs=sE, start=True, stop=False)
                nc.tensor.matmul(out=Y_Le, lhsT=wtile["w_va_eo"][:, :], rhs=sO, start=False, stop=True)
                nc.tensor.matmul(out=Y_Lo, lhsT=wtile["w_va_oe"][:, :], rhs=sE, start=True, stop=False)
                nc.tensor.matmul(out=Y_Lo, lhsT=wtile["w_va_oo"][:, :], rhs=sO, start=False, stop=True)
                Y_hA = ps0.tile([P, G, 255], F32, tag="Y_hA")
                Y_hB = ps0.tile([P, G, 255], F32, tag="Y_hB")
                nc.tensor.matmul(out=Y_hA, lhsT=wtile["w_vh_ae"][:, :], rhs=ch0E, start=True, stop=False)
                nc.tensor.matmul(out=Y_hA, lhsT=wtile["w_vh_ao"][:, :], rhs=ch0O, start=False, stop=True)
                nc.tensor.matmul(out=Y_hB, lhsT=wtile["w_vh_be"][:, :], rhs=ch0E, start=True, stop=False)
                nc.tensor.matmul(out=Y_hB, lhsT=wtile["w_vh_bo"][:, :], rhs=ch0O, start=False, stop=True)

                # ---- level-0 horizontal resizes ----------------------------
                L0e = rh255(Y_Le, "L0e")
                L0o = rh255(Y_Lo, "L0o")
                h0A = rh255(Y_hA, "h0A")
                h0B = rh255(Y_hB, "h0B")

                # ---- level-1 elementwise -----------------------------------
                s2e = lv1.tile([P, G, 128], BF16, tag="s2e")
                s2o = lv1.tile([P, G, 128], BF16, tag="s2o")
                d2e = lv1.tile([P, G, 128], BF16, tag="d2e")
                d2o = lv1.tile([P, G, 128], BF16, tag="d2o")
                nc.vector.tensor_tensor(out=s2e, in0=L0e[:, :, 0:256:2], in1=L0e[:, :, 1:256:2], op=Alu.add)
                nc.vector.tensor_tensor(out=s2o, in0=L0o[:, :, 0:256:2], in1=L0o[:, :, 1:256:2], op=Alu.add)
                nc.vector.tensor_tensor(out=d2e, in0=L0e[:, :, 0:256:2], in1=L0e[:, :, 1:256:2], op=Alu.subtract)
                nc.vector.tensor_tensor(out=d2o, in0=L0o[:, :, 0:256:2], in1=L0o[:, :, 1:256:2], op=Alu.subtract)
                lsum1 = lv1.tile([P, G, 128], BF16, tag="lsum1")
                t1b = lv1.tile([P, G, 128], BF16, tag="t1b")
                ad2e = lv1.tile([P, G, 128], BF16, tag="ad2e")
                ad2o = lv1.tile([P, G, 128], BF16, tag="ad2o")
                m1 = lv1.tile([P, G, 128], BF16, tag="m1")
                nc.vector.tensor_tensor(out=lsum1, in0=s2e, in1=s2o, op=Alu.add)
                nc.vector.tensor_tensor(out=t1b, in0=s2e, in1=s2o, op=Alu.subtract)
                nc.scalar.activation(out=ad2e, in_=d2e, func=Act.Abs)
                nc.scalar.activation(out=ad2o, in_=d2o, func=Act.Abs)
                nc.vector.tensor_tensor(out=m1, in0=ad2e, in1=ad2o, op=Alu.max)
                a1b = lv1.tile([P, G, 128], BF16, tag="a1b")
                nc.scalar.activation(out=a1b, in_=t1b, func=Act.Abs, scale=0.5)
                ch1 = lv1.tile([P, G, 128], BF16, tag="ch1")
                nc.vector.tensor_tensor(out=ch1, in0=a1b, in1=m1, op=Alu.add)

                # ---- level-1 vertical matmuls ------------------------------
                Y_lo = ps1.tile([P, 2, G, 128], F32, tag="Y_lo")
                Y_h1 = ps1.tile([P, 2, G, 128], F32, tag="Y_h1")
                nc.tensor.matmul(out=Y_lo[:, 0], lhsT=wtile["w_vq_a"][:, :], rhs=lsum1, start=True, stop=True)
                nc.tensor.matmul(out=Y_lo[:, 1], lhsT=wtile["w_vq_b"][:, :], rhs=lsum1, start=True, stop=True)
                nc.tensor.matmul(out=Y_h1[:, 0], lhsT=wtile["w_vq_a"][:, :], rhs=ch1, start=True, stop=True)
                nc.tensor.matmul(out=Y_h1[:, 1], lhsT=wtile["w_vq_b"][:, :], rhs=ch1, start=True, stop=True)

                # ---- level-1 horizontal (128->256) + finalization ----------
                def rh128(Y, name):
                    """Y: PSUM [128, 2, G, 128] -> (ev, od) bf16 [128,2,G,128]."""
                    q = lv1.tile([P, 2, G, 129], BF16, tag=f"q_{name}")
                    nc.scalar.copy(out=q[:, :, :, 0:128], in_=Y)
                    nc.scalar.copy(out=q[:, :, :, 128:129], in_=Y[:, :, :, 127:128])
                    q1 = lv1.tile([P, 2, G, 129], BF16, tag=f"q1_{name}")
                    nc.gpsimd.tensor_copy(out=q1[:, :, :, 1:129], in_=q[:, :, :, 0:128])
                    nc.gpsimd.tensor_copy(out=q1[:, :, :, 0:1], in_=q[:, :, :, 0:1])
                    diff = lv1.tile([P, 2, G, 129], BF16, tag=f"df_{name}")
                    nc.vector.tensor_tensor(out=diff, in0=q1, in1=q, op=Alu.subtract)
                    ev = lv1.tile([P, 2, G, 128], BF16, tag=f"ev_{name}")
                    od = lv1.tile([P, 2, G, 128], BF16, tag=f"od_{name}")
                    nc.vector.scalar_tensor_tensor(
                        out=ev, in0=diff[:, :, :, 0:128], scalar=0.25,
                        in1=q[:, :, :, 0:128], op0=Alu.mult, op1=Alu.add)
                    nc.vector.scalar_tensor_tensor(
                        out=od, in0=diff[:, :, :, 1:129], scalar=-0.25,
                        in1=q[:, :, :, 0:128], op0=Alu.mult, op1=Alu.add)
                    return ev, od

                lo_ev, lo_od = rh128(Y_lo, "lo")
                h1_ev, h1_od = rh128(Y_h1, "h1")

                lowA = outp.tile([P, G, W_], F32, tag="lowA")
                lowB = outp.tile([P, G, W_], F32, tag="lowB")
                nc.gpsimd.tensor_copy(out=lowA[:, :, 0:256:2], in_=lo_ev[:, 0])
                nc.gpsimd.tensor_copy(out=lowA[:, :, 1:256:2], in_=lo_od[:, 0])
                nc.gpsimd.tensor_copy(out=lowB[:, :, 0:256:2], in_=lo_ev[:, 1])
                nc.gpsimd.tensor_copy(out=lowB[:, :, 1:256:2], in_=lo_od[:, 1])

                highA = outp.tile([P, G, W_], F32, tag="highA")
                highB = outp.tile([P, G, W_], F32, tag="highB")
                nc.vector.tensor_tensor(out=highA[:, :, 0:256:2], in0=h1_ev[:, 0],
                                        in1=h0A[:, :, 0:256:2], op=Alu.add)
                nc.vector.tensor_tensor(out=highA[:, :, 1:256:2], in0=h1_od[:, 0],
                                        in1=h0A[:, :, 1:256:2], op=Alu.add)
                nc.vector.tensor_tensor(out=highB[:, :, 0:256:2], in0=h1_ev[:, 1],
                                        in1=h0B[:, :, 0:256:2], op=Alu.add)
                nc.vector.tensor_tensor(out=highB[:, :, 1:256:2], in0=h1_od[:, 1],
                                        in1=h0B[:, :, 1:256:2], op=Alu.add)

                # ---- store --------------------------------------------------
                nc.sync.dma_start(
                    out=low_d[c0:c0 + G, 0:128, :].rearrange("c r w -> r c w"), in_=lowA)
                nc.sync.dma_start(
                    out=low_d[c0:c0 + G, 128:256, :].rearrange("c r w -> r c w"), in_=lowB)
                nc.sync.dma_start(
                    out=high_d[c0:c0 + G, 0:128, :].rearrange("c r w -> r c w"), in_=highA)
                nc.sync.dma_start(
                    out=high_d[c0:c0 + G, 128:256, :].rearrange("c r w -> r c w"), in_=highB)

    nc.compile()
    _NC_CACHE[key] = nc
    return nc


# ----------------------------------------------------------------------------
# host entry points
# ----------------------------------------------------------------------------

_RUNNER = None


def _get_runner():
    """Builds (once) a cached sharded jit executable over the 8 cores.

    Mirrors bass2jax.run_bass_via_pjrt's multi-core path, but without
    donation (the kernel writes every output element, so output buffers
    need not be zero-shipped per call) and with the jitted callable plus
    the device-resident weight/output operands cached across calls.
    """
    global _RUNNER
    if _RUNNER is not None:
        return _RUNNER

    import jax
    from jax.sharding import Mesh, PartitionSpec, NamedSharding
    from jax.experimental.shard_map import shard_map
    import concourse.mybir as mybir
    from concourse import bass2jax
    from concourse.bass2jax import _bass_exec_p, partition_id_tensor

    bass2jax.install_neuronx_cc_hook()
    nc = build_nc(C_)

    partition_name = nc.partition_id_tensor.name if nc.partition_id_tensor else None
    in_names, out_names, out_avals = [], [], []
    for alloc in nc.m.functions[0].allocations:
        if not isinstance(alloc, mybir.MemoryLocationSet):
            continue
        name = alloc.memorylocations[0].name
        if alloc.kind == "ExternalInput":
            if name != partition_name:
                in_names.append(name)
        elif alloc.kind == "ExternalOutput":
            out_names.append(name)
            out_avals.append(jax.core.ShapedArray(
                tuple(alloc.tensor_shape), mybir.dt.np(alloc.dtype)))
    n_params = len(in_names)
    all_in_names = list(in_names) + list(out_names)
    if partition_name is not None:
        all_in_names.append(partition_name)

    def _body(*args):
        operands = list(args)
        if partition_name is not None:
            operands.append(partition_id_tensor())
        return tuple(_bass_exec_p.bind(
            *operands,
            out_avals=tuple(out_avals),
            in_names=tuple(all_in_names),
            out_names=tuple(out_names),
            lowering_input_output_aliases=(),
            sim_require_finite=True,
            sim_require_nnan=True,
            nc=nc,
        ))

    devices = jax.devices()[:NCORES]
    mesh = Mesh(np.asarray(devices), ("core",))
    n_in = n_params + len(out_names)
    sharded = jax.jit(shard_map(
        _body, mesh=mesh,
        in_specs=(PartitionSpec("core"),) * n_in,
        out_specs=(PartitionSpec("core"),) * len(out_names),
        check_rep=False))

    shard0 = NamedSharding(mesh, PartitionSpec("core"))
    wt = _weights()
    # device-resident static operands: weights (replicated per core) and
    # uninitialized-output placeholders
    static = {}
    for name in in_names:
        if name == "x":
            continue
        arr = np.concatenate([wt[name]] * NCORES, axis=0)
        static[name] = jax.device_put(arr, shard0)
    for name, aval in zip(out_names, out_avals):
        z = np.zeros((aval.shape[0] * NCORES,) + tuple(aval.shape[1:]),
                     dtype=aval.dtype)
        static[name] = jax.device_put(z, shard0)

    def run(x_global):
        """x_global: np or jax array [8*64, 256, 256] fp32 (sharded ok)."""
        ops = []
        for name in in_names:
            ops.append(x_global if name == "x" else static[name])
        for name in out_names:
            ops.append(static[name])
        outs = sharded(*ops)
        return dict(zip(out_names, outs))

    _RUNNER = (run, shard0)
    return _RUNNER


def _run_device(x, trace=False):
    """x: [8, 64, 256, 256] fp32. Returns (low, high, results_obj)."""
    if trace:
        from concourse import bass_utils
        nc = build_nc(C_)
        wt = _weights()
        in_maps = [dict(wt, x=np.ascontiguousarray(x[b])) for b in range(NCORES)]
        res = bass_utils.run_bass_kernel_spmd(
            nc, in_maps, core_ids=list(range(NCORES)), trace=True)
        low = np.stack([res.results[b]["low"] for b in range(NCORES)])
        high = np.stack([res.results[b]["high"] for b in range(NCORES)])
        return low, high, res

    run, _ = _get_runner()
    outs = run(np.ascontiguousarray(x).reshape(B_ * C_, H_, W_))
    low = np.asarray(outs["low"]).reshape(B_, C_, H_, W_)
    high = np.asarray(outs["high"]).reshape(B_, C_, H_, W_)
    return low, high, None


def _fallback(x, level):
    """Numpy port of the reference for unexpected shapes/levels."""
    xl = x.astype(np.float64)
    Bb, Cc, H, W = xl.shape
    low = xl
    high = np.zeros_like(xl)

    def up(a, n_r, n_c):
        Mr = _resize_matrix(a.shape[-2], n_r)
        Mc = _resize_matrix(a.shape[-1], n_c)
        return np.einsum("ij,...jk,lk->...il", Mr, a, Mc)

    for lv in range(level):
        stride = 2 ** lv
        if H // stride < 2 or W // stride < 2:
            break
        x00 = low[..., 0:H - 1:stride, 0:W - 1:stride]
        x01 = low[..., 0:H - 1:stride, 1:W:stride]
        x10 = low[..., 1:H:stride, 0:W - 1:stride]
        x11 = low[..., 1:H:stride, 1:W:stride]
        ll = (x00 + x01 + x10 + x11) * 0.25
        lh = (x00 + x01 - x10 - x11) * 0.25
        hl = (x00 - x01 + x10 - x11) * 0.25
        hh = (x00 - x01 - x10 + x11) * 0.25
        ch = np.abs(lh) + np.abs(hl) + np.abs(hh)
        high = high + up(ch, H, W)
        low = up(ll, H, W)
    if level > 0:
        high = high / level
    return low.astype(np.float32), high.astype(np.float32)


def kernel(x, level):
    x = np.asarray(x, dtype=np.float32)
    level = int(level)
    if level != 2 or x.shape != (B_, C_, H_, W_):
        return _fallback(x, level)
    low, high, _ = _run_device(x)
    return low, high
